# revision 6
# baseline (speedup 1.0000x reference)
"""Trainium2 Bass kernel for the BiLSTM-CRF loss (sum reduction).

Strategy (v3):
- Data-parallel: batch 256 sharded as 32 per NeuronCore across 8 cores.
- Host passes emissions pre-transposed to [T, (step j, seg k, batch b)] and
  pre-cast to bf16 (layout/dtype prep only; all math stays on device). This
  removes all on-device PE transposes and DVE casts and halves HBM traffic.
- Normalizer (forward algorithm) runs in LINEAR space: alpha_{s+1} =
  exp(em_{s+1}) .* (E^T alpha_s) with E = exp(transitions); each step is a
  PE matmul plus one elementwise DVE multiply (PSUM f32 x SBUF bf16).
- The 511-step serial chain is cut ~12x by exploiting the Birkhoff
  contraction of E: 16 segments run as concurrent chains (one [128,512]
  matmul round split in two halves), interior segments converge from a
  uniform vector during 6 burn-in rounds. Per-segment growth is accounted
  via boundary column sums; fp32 range kept by 5 delayed column rescales.
- em chunks stream via HWDGE DMA in consumption order; ACT exponentiates
  each chunk just ahead of the chain.
- Numerator: two indirect-DMA element gathers + reductions, overlapped.

kernel() contract: full unsharded inputs in, full output (scalar) out.
"""
import numpy as np
import ml_dtypes

S, B, T = 512, 256, 128
NCORES, Bl = 8, 32
NSEG, BURN = 16, 6
NR = BURN + 32                       # 38 rounds
H = NSEG // 2
RESC_APPLY = [BURN + 3, BURN + 9, BURN + 15, BURN + 21, BURN + 27]
C_RESC = 2.0 ** -46                  # constant column rescale factor
RESC_LOGSUM = len(RESC_APPLY) * 46 * float(np.log(2.0))
INIT_BURN = 2.0 ** -30
TSSE_N = T * T + T + T + 1           # 16641: trans | start | end | 0.0
TSSE_PAD = TSSE_N - 1                # index of the 0.0 entry
NCOL = 32 * NSEG * Bl                # 16384 em columns (j, k, b)
CH = 1024                            # em chunk width (2 j-slabs)

_NC = None


def _build():
    import concourse.bass as bass
    import concourse.tile as tile
    from concourse import bacc, mybir
    from contextlib import ExitStack

    f32 = mybir.dt.float32
    bf16 = mybir.dt.bfloat16
    i32 = mybir.dt.int32
    AF = mybir.ActivationFunctionType
    OP = mybir.AluOpType
    AX = mybir.AxisListType

    nc = bacc.Bacc("TRN2", target_bir_lowering=False, debug=False,
                   num_devices=NCORES)

    emT = nc.dram_tensor("emT", [T, NCOL], bf16, kind="ExternalInput")
    emg = nc.dram_tensor("emg", [S * Bl, T], f32, kind="ExternalInput")
    transm = nc.dram_tensor("transm", [T, T], f32, kind="ExternalInput")
    startv = nc.dram_tensor("startv", [T, 1], f32, kind="ExternalInput")
    endv = nc.dram_tensor("endv", [T, 1], f32, kind="ExternalInput")
    emtidx = nc.dram_tensor("emtidx", [128, 128], i32, kind="ExternalInput")
    tssev = nc.dram_tensor("tssev", [TSSE_N, 1], f32, kind="ExternalInput")
    tsseidx = nc.dram_tensor("tsseidx", [128, 129], i32, kind="ExternalInput")
    outv = nc.dram_tensor("out", [1, 1], f32, kind="ExternalOutput")

    with tile.TileContext(nc) as tc, ExitStack() as ctx:
        const = ctx.enter_context(tc.tile_pool(name="const", bufs=1))
        pchain = ctx.enter_context(tc.tile_pool(name="pchain", bufs=2,
                                                space="PSUM"))
        pstat = ctx.enter_context(tc.tile_pool(name="pstat", bufs=2,
                                               space="PSUM"))

        # ---------- constants ----------
        ones_col = const.tile([128, 1], bf16)
        nc.vector.memset(ones_col[:], 1.0)
        ones_colf = const.tile([128, 1], f32)
        nc.vector.memset(ones_colf[:], 1.0)

        tr_sb = const.tile([128, 128], f32)
        nc.sync.dma_start(out=tr_sb[:], in_=transm[:, :])
        E_f = const.tile([128, 128], f32)
        nc.scalar.activation(E_f[:], tr_sb[:], AF.Exp)
        E_hi = const.tile([128, 128], bf16)
        nc.vector.tensor_copy(out=E_hi[:], in_=E_f[:])
        st_sb = const.tile([128, 1], f32)
        nc.sync.dma_start(out=st_sb[:], in_=startv[:, :])
        Estart = const.tile([128, 1], f32)
        nc.scalar.activation(Estart[:], st_sb[:], AF.Exp)
        en_sb = const.tile([128, 1], f32)
        nc.sync.dma_start(out=en_sb[:], in_=endv[:, :])
        Eend = const.tile([128, 1], bf16)
        nc.scalar.activation(Eend[:], en_sb[:], AF.Exp)

        # ---------- numerator: indirect gathers + reductions ----------
        emtidx_sb = const.tile([128, 128], i32)
        nc.sync.dma_start(out=emtidx_sb[:], in_=emtidx[:, :])
        tsseidx_sb = const.tile([128, 129], i32)
        nc.sync.dma_start(out=tsseidx_sb[:], in_=tsseidx[:, :])
        gem = const.tile([128, 128], f32)
        nc.gpsimd.indirect_dma_start(
            out=gem[:], out_offset=None,
            in_=bass.AP(tensor=emg, offset=0,
                        ap=[[1, S * Bl * T], [1, 1]]),
            in_offset=bass.IndirectOffsetOnAxis(ap=emtidx_sb[:], axis=0))
        gts = const.tile([128, 129], f32)
        nc.gpsimd.indirect_dma_start(
            out=gts[:], out_offset=None,
            in_=bass.AP(tensor=tssev, offset=0,
                        ap=[[1, TSSE_N], [1, 1]]),
            in_offset=bass.IndirectOffsetOnAxis(ap=tsseidx_sb[:], axis=0))

        # ---------- chain state + emission storage ----------
        A = const.tile([128, NSEG, Bl], bf16)
        nc.vector.memset(A[:], INIT_BURN)
        A2 = A.rearrange("p k b -> p (k b)")
        emsb = const.tile([128, NCOL], bf16)
        erm = const.tile([128, NCOL], bf16)

        n_sb = const.tile([1, NSEG * Bl], f32)
        m_sb = const.tile([1, NSEG * Bl], f32)
        fin_sb = const.tile([1, Bl], f32)

        def dma_chunk(c):
            nc.sync.dma_start(out=emsb[:, CH * c:CH * (c + 1)],
                              in_=emT[:, CH * c:CH * (c + 1)])

        def exp_chunk(c):
            nc.scalar.activation(erm[:, CH * c:CH * (c + 1)],
                                 emsb[:, CH * c:CH * (c + 1)], AF.Exp)

        def erm_off(r, ka):
            # column offset into erm for round r, half starting at segment ka
            if r < BURN - 1:            # replay steps of previous segment
                return (r + 32 - BURN + 1) * 512 + (ka - 1) * Bl
            if r == BURN - 1:           # s = 32k rows (j=0, seg k)
                return ka * Bl
            c = r - BURN + 1            # own-segment step 1..32
            if c <= 31:
                return c * 512 + ka * Bl
            return (ka + 1) * Bl        # step 32 = (j=0, seg k+1)

        def emit_round(r):
            if r < BURN:
                ksl = [(1, H), (H, NSEG)]
            elif r < NR - 1:
                ksl = [(0, H), (H, NSEG)]
            else:
                ksl = [(0, H), (H, NSEG - 1)]
            for (ka, kb), tg in zip(ksl, ("psA", "psB")):
                w = (kb - ka) * Bl
                ps = pchain.tile([128, H * Bl], f32, tag=tg)
                nc.tensor.matmul(out=ps[:, :w], lhsT=E_hi[:],
                                 rhs=A2[:, ka * Bl:kb * Bl],
                                 start=True, stop=True)
                off = erm_off(r, ka)
                nc.vector.tensor_tensor(
                    out=A2[:, ka * Bl:kb * Bl], in0=ps[:, :w],
                    in1=erm[:, off:off + w], op=OP.mult)
            if r in RESC_APPLY:
                nc.vector.tensor_scalar_mul(A2[:], A2[:], C_RESC)
            if r == BURN - 1:
                cs = pstat.tile([1, NSEG * Bl], f32, tag="st")
                nc.tensor.matmul(out=cs[:], lhsT=ones_col[:], rhs=A2[:],
                                 start=True, stop=True)
                nc.vector.tensor_copy(out=n_sb[:], in_=cs[:])
            if r == NR - 2:
                m15 = pstat.tile([1, NSEG * Bl], f32, tag="st")
                nc.tensor.matmul(out=m15[:, :Bl], lhsT=ones_col[:],
                                 rhs=A2[:, (NSEG - 1) * Bl:],
                                 start=True, stop=True)
                nc.vector.tensor_copy(out=m_sb[:, (NSEG - 1) * Bl:],
                                      in_=m15[:, :Bl])
                fin = pstat.tile([1, NSEG * Bl], f32, tag="st")
                nc.tensor.matmul(out=fin[:, :Bl], lhsT=Eend[:],
                                 rhs=A2[:, (NSEG - 1) * Bl:],
                                 start=True, stop=True)
                nc.vector.tensor_copy(out=fin_sb[:], in_=fin[:, :Bl])
            if r == NR - 1:
                mm = pstat.tile([1, NSEG * Bl], f32, tag="st")
                nc.tensor.matmul(out=mm[:, :(NSEG - 1) * Bl],
                                 lhsT=ones_col[:],
                                 rhs=A2[:, :(NSEG - 1) * Bl],
                                 start=True, stop=True)
                nc.vector.tensor_copy(out=m_sb[:, :(NSEG - 1) * Bl],
                                      in_=mm[:, :(NSEG - 1) * Bl])

        # ---------- emission pipeline + chain ----------
        for c in (13, 14, 15, 0):
            dma_chunk(c)
        for c in range(1, 13):
            dma_chunk(c)
        exp_chunk(13)
        exp_chunk(14)
        exp_chunk(15)
        for r in range(BURN - 1):
            emit_round(r)
        exp_chunk(0)
        nc.vector.tensor_scalar_mul(A[:, 0, :], erm[:, 0:Bl], Estart[:])
        emit_round(BURN - 1)
        for r in range(BURN, NR):
            j = r - BURN + 1
            if j >= 2 and j % 2 == 0 and j // 2 <= 12:
                exp_chunk(j // 2)
            emit_round(r)

        # ---------- final assembly ----------
        gsum1 = const.tile([128, 1], f32)
        nc.vector.reduce_sum(out=gsum1[:], in_=gem[:], axis=AX.X)
        gsum2 = const.tile([128, 1], f32)
        nc.vector.reduce_sum(out=gsum2[:], in_=gts[:], axis=AX.X)
        numcol = const.tile([128, 1], f32)
        nc.vector.tensor_add(out=numcol[:], in0=gsum1[:], in1=gsum2[:])
        logn = const.tile([1, NSEG * Bl], f32)
        nc.scalar.activation(logn[:], n_sb[:], AF.Ln)
        logm = const.tile([1, NSEG * Bl], f32)
        nc.scalar.activation(logm[:], m_sb[:], AF.Ln)
        grow = const.tile([1, NSEG * Bl], f32)
        nc.vector.tensor_tensor(out=grow[:], in0=logm[:], in1=logn[:],
                                op=OP.subtract)
        nc.vector.tensor_scalar_add(grow[:], grow[:], RESC_LOGSUM)
        growb = const.tile([1, Bl], f32)
        nc.vector.reduce_sum(out=growb[:],
                             in_=grow.rearrange("p (k b) -> p b k", k=NSEG),
                             axis=AX.X)
        logfin = const.tile([1, Bl], f32)
        nc.scalar.activation(logfin[:], fin_sb[:], AF.Ln)
        lz = const.tile([1, Bl], f32)
        nc.vector.tensor_add(out=lz[:], in0=growb[:], in1=logfin[:])
        nc.vector.tensor_tensor(out=lz[:], in0=lz[:],
                                in1=logm[:, (NSEG - 1) * Bl:], op=OP.subtract)
        nc.vector.tensor_add(out=lz[:], in0=lz[:], in1=logn[:, :Bl])
        lzs = const.tile([1, 1], f32)
        nc.vector.reduce_sum(out=lzs[:], in_=lz[:], axis=AX.X)
        nps = pstat.tile([1, NSEG * Bl], f32, tag="st")
        nc.tensor.matmul(out=nps[:, :1], lhsT=ones_colf[:], rhs=numcol[:],
                         start=True, stop=True)
        res = const.tile([1, 1], f32)
        nc.vector.tensor_tensor(out=res[:], in0=nps[:, :1], in1=lzs[:],
                                op=OP.subtract)
        nc.sync.dma_start(out=outv[:, :], in_=res[:])

    nc.compile()
    return nc


def _get_nc():
    global _NC
    if _NC is None:
        _NC = _build()
    return _NC


def make_in_maps(inputs):
    em = np.asarray(inputs["emissions"], dtype=np.float32)
    tags = np.asarray(inputs["tags"]).astype(np.int32)
    st = np.asarray(inputs["start_transitions"], dtype=np.float32)
    en = np.asarray(inputs["end_transitions"], dtype=np.float32)
    tr = np.ascontiguousarray(np.asarray(inputs["transitions"],
                                         dtype=np.float32))
    tssev = np.concatenate(
        [tr.ravel(), st, en, np.zeros(1, np.float32)]).astype(
        np.float32).reshape(TSSE_N, 1)
    s_i = np.arange(S)[:, None]
    b_i = np.arange(Bl)[None, :]
    in_maps = []
    for c in range(NCORES):
        em_c = em[:, c * Bl:(c + 1) * Bl, :]
        # [s=(k,j), b, t] -> [t, j, k, b]; col = j*512 + k*32 + b
        emr = np.ascontiguousarray(
            em_c.reshape(NSEG, 32, Bl, T).transpose(3, 1, 0, 2)
        ).reshape(T, NCOL)
        emT_b = emr.astype(ml_dtypes.bfloat16)
        tg = tags[:, c * Bl:(c + 1) * Bl]
        emi = ((s_i * Bl + b_i) * T + tg).astype(np.int32).reshape(128, 128)
        tse = np.full(128 * 129, TSSE_PAD, np.int32)
        tse[:511 * Bl] = (tg[:-1] * T + tg[1:]).astype(np.int32).ravel()
        tse[511 * Bl:511 * Bl + Bl] = T * T + tg[0]
        tse[511 * Bl + Bl:511 * Bl + 2 * Bl] = T * T + T + tg[-1]
        in_maps.append({
            "emT": emT_b,
            "emg": np.ascontiguousarray(em_c).reshape(S * Bl, T),
            "transm": tr,
            "startv": st.reshape(T, 1),
            "endv": en.reshape(T, 1),
            "emtidx": emi,
            "tssev": tssev,
            "tsseidx": tse.reshape(128, 129),
        })
    return in_maps


def _numpy_fallback(inputs):
    """Exact float64 port of the reference (handles arbitrary masks)."""
    em = np.asarray(inputs["emissions"], dtype=np.float64)
    tags = np.asarray(inputs["tags"]).astype(np.int64)
    mask = np.asarray(inputs["mask"]).astype(bool)
    st = np.asarray(inputs["start_transitions"], dtype=np.float64)
    en = np.asarray(inputs["end_transitions"], dtype=np.float64)
    tr = np.asarray(inputs["transitions"], dtype=np.float64)
    Sl, Bn = tags.shape
    mask_f = mask.astype(np.float64)
    emit = np.take_along_axis(em, tags[:, :, None], axis=2)[:, :, 0]
    trsc = tr[tags[:-1], tags[1:]]
    score = st[tags[0]] + emit[0]
    score = score + ((trsc + emit[1:]) * mask_f[1:]).sum(0)
    seq_ends = mask.astype(np.int64).sum(0) - 1
    score = score + en[tags[seq_ends, np.arange(Bn)]]
    alpha = st[None, :] + em[0]
    for s in range(1, Sl):
        nxt = alpha[:, :, None] + tr[None] + em[s][:, None, :]
        mx = nxt.max(axis=1)
        nxt = mx + np.log(np.exp(nxt - mx[:, None, :]).sum(axis=1))
        alpha = np.where(mask[s][:, None], nxt, alpha)
    z = alpha + en[None, :]
    mz = z.max(axis=1)
    logZ = mz + np.log(np.exp(z - mz[:, None]).sum(axis=1))
    return np.asarray((score - logZ).sum(), dtype=np.float32)


def run_device(inputs, trace=False, trace_kwargs=None):
    from concourse.bass_utils import run_bass_kernel_spmd
    nc = _get_nc()
    in_maps = make_in_maps(inputs)
    br = run_bass_kernel_spmd(nc, in_maps, list(range(NCORES)),
                              trace=trace, **(trace_kwargs or {}))
    total = np.float32(
        sum(float(br.results[i]["out"][0, 0]) for i in range(NCORES)))
    return np.asarray(total, dtype=np.float32), br


def kernel(**inputs):
    mask = np.asarray(inputs["mask"])
    if not bool(mask.all()):
        return _numpy_fallback(inputs)
    val, _ = run_device(inputs, trace=False)
    return val


# revision 7
# speedup vs baseline: 1.0914x; 1.0914x over previous
"""Trainium2 Bass kernel for the BiLSTM-CRF loss (sum reduction).

Strategy (v4):
- Data-parallel: batch 256 sharded as 32 per NeuronCore across 8 cores.
- Host passes emissions pre-transposed to [T, (step j, seg k, batch b)] and
  pre-cast to bf16 (layout/dtype prep only; all math stays on device), plus
  a f32 copy for the exact numerator gather. This removes all on-device PE
  transposes and DVE casts and halves streamed HBM traffic.
- Normalizer (forward algorithm) runs in LINEAR space: alpha_{s+1} =
  exp(em_{s+1}) .* (E^T alpha_s) with E = exp(transitions); each step is a
  PE matmul plus one elementwise DVE multiply (PSUM f32 x SBUF bf16).
- The 511-step serial chain is cut ~24x: 32 segments of 16 steps run as
  concurrent chains (one [128,1024] matmul round split in two halves);
  interior segments converge from a uniform vector during 5 burn-in rounds
  (Birkhoff contraction ~0.1/step). Per-segment growth is accounted via
  boundary column sums; fp32 range kept by 2 column rescales folded into
  the round multiply (scalar_tensor_tensor).
- em j-slabs stream via HWDGE DMA in consumption order; ACT exponentiates
  each slab just ahead of the chain. Constants go on the scalar HWDGE ring
  so descriptor generation overlaps.
- Numerator: two indirect-DMA element gathers + reductions, overlapped.

kernel() contract: full unsharded inputs in, full output (scalar) out.
"""
import numpy as np
import ml_dtypes

S, B, T = 512, 256, 128
NCORES, Bl = 8, 32
NSEG, BURN = 32, 5
SEGL = S // NSEG                     # 16 steps per segment
NR = BURN + SEGL                     # 21 rounds
H = NSEG // 2
RESC_APPLY = [BURN + 3, BURN + 9]
C_RESC = 2.0 ** -46                  # constant column rescale factor
RESC_LOGSUM = len(RESC_APPLY) * 46 * float(np.log(2.0))
INIT_BURN = 2.0 ** -30
TSSE_N = T * T + T + T + 1           # 16641: trans | start | end | 0.0
TSSE_PAD = TSSE_N - 1                # index of the 0.0 entry
NCOL = SEGL * NSEG * Bl              # 16384 em columns (j, k, b)
SLAB = NSEG * Bl                     # 1024 columns per j-slab

_NC = None


def _build():
    import concourse.bass as bass
    import concourse.tile as tile
    from concourse import bacc, mybir
    from contextlib import ExitStack

    f32 = mybir.dt.float32
    bf16 = mybir.dt.bfloat16
    i32 = mybir.dt.int32
    AF = mybir.ActivationFunctionType
    OP = mybir.AluOpType
    AX = mybir.AxisListType

    nc = bacc.Bacc("TRN2", target_bir_lowering=False, debug=False,
                   num_devices=NCORES)

    emT = nc.dram_tensor("emT", [T, NCOL], bf16, kind="ExternalInput")
    emg = nc.dram_tensor("emg", [S * Bl, T], f32, kind="ExternalInput")
    transm = nc.dram_tensor("transm", [T, T], f32, kind="ExternalInput")
    startv = nc.dram_tensor("startv", [T, 1], f32, kind="ExternalInput")
    endv = nc.dram_tensor("endv", [T, 1], f32, kind="ExternalInput")
    emtidx = nc.dram_tensor("emtidx", [128, 128], i32, kind="ExternalInput")
    tssev = nc.dram_tensor("tssev", [TSSE_N, 1], f32, kind="ExternalInput")
    tsseidx = nc.dram_tensor("tsseidx", [128, 129], i32, kind="ExternalInput")
    outv = nc.dram_tensor("out", [1, 1], f32, kind="ExternalOutput")

    with tile.TileContext(nc) as tc, ExitStack() as ctx:
        const = ctx.enter_context(tc.tile_pool(name="const", bufs=1))
        pchain = ctx.enter_context(tc.tile_pool(name="pchain", bufs=2,
                                                space="PSUM"))
        pstat = ctx.enter_context(tc.tile_pool(name="pstat", bufs=2,
                                               space="PSUM"))

        # ---------- em slab DMAs first (sync HWDGE ring) ----------
        emsb = const.tile([128, NCOL], bf16)

        def dma_slab(j):
            nc.sync.dma_start(out=emsb[:, SLAB * j:SLAB * (j + 1)],
                              in_=emT[:, SLAB * j:SLAB * (j + 1)])

        slab_order = [SEGL - 4, SEGL - 3, SEGL - 2, SEGL - 1, 0] + \
            list(range(1, SEGL - 4))
        for j in slab_order:
            dma_slab(j)

        # ---------- constants (scalar HWDGE ring, overlapped) ----------
        emtidx_sb = const.tile([128, 128], i32)
        nc.scalar.dma_start(out=emtidx_sb[:], in_=emtidx[:, :])
        tsseidx_sb = const.tile([128, 129], i32)
        nc.scalar.dma_start(out=tsseidx_sb[:], in_=tsseidx[:, :])
        tr_sb = const.tile([128, 128], f32)
        nc.scalar.dma_start(out=tr_sb[:], in_=transm[:, :])
        st_sb = const.tile([128, 1], f32)
        nc.scalar.dma_start(out=st_sb[:], in_=startv[:, :])
        en_sb = const.tile([128, 1], f32)
        nc.scalar.dma_start(out=en_sb[:], in_=endv[:, :])

        ones_col = const.tile([128, 1], bf16)
        nc.vector.memset(ones_col[:], 1.0)
        ones_colf = const.tile([128, 1], f32)
        nc.vector.memset(ones_colf[:], 1.0)

        # ---------- numerator: indirect gathers (gpsimd SWDGE) ----------
        gem = const.tile([128, 128], f32)
        nc.gpsimd.indirect_dma_start(
            out=gem[:], out_offset=None,
            in_=bass.AP(tensor=emg, offset=0,
                        ap=[[1, S * Bl * T], [1, 1]]),
            in_offset=bass.IndirectOffsetOnAxis(ap=emtidx_sb[:], axis=0))
        gts = const.tile([128, 129], f32)
        nc.gpsimd.indirect_dma_start(
            out=gts[:], out_offset=None,
            in_=bass.AP(tensor=tssev, offset=0,
                        ap=[[1, TSSE_N], [1, 1]]),
            in_offset=bass.IndirectOffsetOnAxis(ap=tsseidx_sb[:], axis=0))

        # ---------- chain state ----------
        erm = const.tile([128, NCOL], bf16)
        A = const.tile([128, NSEG, Bl], bf16)
        nc.vector.memset(A[:], INIT_BURN)
        A2 = A.rearrange("p k b -> p (k b)")

        n_sb = const.tile([1, NSEG * Bl], f32)
        m_sb = const.tile([1, NSEG * Bl], f32)
        fin_sb = const.tile([1, Bl], f32)

        def exp_slab(j):
            nc.scalar.activation(erm[:, SLAB * j:SLAB * (j + 1)],
                                 emsb[:, SLAB * j:SLAB * (j + 1)], AF.Exp)

        def erm_off(r, ka):
            # column offset into erm for round r, half starting at segment ka
            if r < BURN - 1:            # replay steps of previous segment
                return (r + SEGL - BURN + 1) * SLAB + (ka - 1) * Bl
            if r == BURN - 1:           # s = SEGL*k rows (j=0, seg k)
                return ka * Bl
            c = r - BURN + 1            # own-segment step 1..SEGL
            if c <= SEGL - 1:
                return c * SLAB + ka * Bl
            return (ka + 1) * Bl        # step SEGL = (j=0, seg k+1)

        def colsum(dst, lo, hi):
            # dst[0, lo:hi] = column sums of A2[:, lo:hi] (hi-lo <= 512)
            ps = pstat.tile([1, 512], f32, tag="st")
            nc.tensor.matmul(out=ps[:, :hi - lo], lhsT=ones_col[:],
                             rhs=A2[:, lo:hi], start=True, stop=True)
            nc.vector.tensor_copy(out=dst[:, lo:hi], in_=ps[:, :hi - lo])

        def emit_round(r):
            if r < BURN:
                ksl = [(1, H), (H, NSEG)]
            elif r < NR - 1:
                ksl = [(0, H), (H, NSEG)]
            else:
                ksl = [(0, H), (H, NSEG - 1)]
            for (ka, kb), tg in zip(ksl, ("psA", "psB")):
                w = (kb - ka) * Bl
                ps = pchain.tile([128, H * Bl], f32, tag=tg)
                nc.tensor.matmul(out=ps[:, :w], lhsT=E_hi[:],
                                 rhs=A2[:, ka * Bl:kb * Bl],
                                 start=True, stop=True)
                off = erm_off(r, ka)
                if r in RESC_APPLY:
                    nc.vector.scalar_tensor_tensor(
                        out=A2[:, ka * Bl:kb * Bl], in0=ps[:, :w],
                        scalar=C_RESC, in1=erm[:, off:off + w],
                        op0=OP.mult, op1=OP.mult)
                else:
                    nc.vector.tensor_tensor(
                        out=A2[:, ka * Bl:kb * Bl], in0=ps[:, :w],
                        in1=erm[:, off:off + w], op=OP.mult)
            if r == BURN - 1:
                colsum(n_sb, 0, 512)
                colsum(n_sb, 512, 1024)
            if r == NR - 2:
                m15 = pstat.tile([1, 512], f32, tag="st")
                nc.tensor.matmul(out=m15[:, :Bl], lhsT=ones_col[:],
                                 rhs=A2[:, (NSEG - 1) * Bl:],
                                 start=True, stop=True)
                nc.vector.tensor_copy(out=m_sb[:, (NSEG - 1) * Bl:],
                                      in_=m15[:, :Bl])
                fin = pstat.tile([1, 512], f32, tag="st")
                nc.tensor.matmul(out=fin[:, :Bl], lhsT=Eend[:],
                                 rhs=A2[:, (NSEG - 1) * Bl:],
                                 start=True, stop=True)
                nc.vector.tensor_copy(out=fin_sb[:], in_=fin[:, :Bl])
            if r == NR - 1:
                colsum(m_sb, 0, 512)
                colsum(m_sb, 512, (NSEG - 1) * Bl)

        # ---------- emission pipeline + chain ----------
        exp_slab(SEGL - 4)
        E_f = const.tile([128, 128], f32)
        nc.scalar.activation(E_f[:], tr_sb[:], AF.Exp)
        E_hi = const.tile([128, 128], bf16)
        nc.vector.tensor_copy(out=E_hi[:], in_=E_f[:])
        Estart = const.tile([128, 1], f32)
        nc.scalar.activation(Estart[:], st_sb[:], AF.Exp)
        Eend = const.tile([128, 1], bf16)
        nc.scalar.activation(Eend[:], en_sb[:], AF.Exp)
        for j in (SEGL - 3, SEGL - 2, SEGL - 1, 0):
            exp_slab(j)
        for r in range(BURN - 1):
            emit_round(r)
        nc.vector.tensor_scalar_mul(A[:, 0, :], erm[:, 0:Bl], Estart[:])
        emit_round(BURN - 1)
        for r in range(BURN, NR):
            j = r - BURN + 1
            if 1 <= j <= SEGL - 5:
                exp_slab(j)
            emit_round(r)

        # ---------- final assembly ----------
        gsum1 = const.tile([128, 1], f32)
        nc.vector.reduce_sum(out=gsum1[:], in_=gem[:], axis=AX.X)
        gsum2 = const.tile([128, 1], f32)
        nc.vector.reduce_sum(out=gsum2[:], in_=gts[:], axis=AX.X)
        numcol = const.tile([128, 1], f32)
        nc.vector.tensor_add(out=numcol[:], in0=gsum1[:], in1=gsum2[:])
        logn = const.tile([1, NSEG * Bl], f32)
        nc.scalar.activation(logn[:], n_sb[:], AF.Ln)
        logm = const.tile([1, NSEG * Bl], f32)
        nc.scalar.activation(logm[:], m_sb[:], AF.Ln)
        grow = const.tile([1, NSEG * Bl], f32)
        nc.vector.tensor_tensor(out=grow[:], in0=logm[:], in1=logn[:],
                                op=OP.subtract)
        nc.vector.tensor_scalar_add(grow[:], grow[:], RESC_LOGSUM)
        growb = const.tile([1, Bl], f32)
        nc.vector.reduce_sum(out=growb[:],
                             in_=grow.rearrange("p (k b) -> p b k", k=NSEG),
                             axis=AX.X)
        logfin = const.tile([1, Bl], f32)
        nc.scalar.activation(logfin[:], fin_sb[:], AF.Ln)
        lz = const.tile([1, Bl], f32)
        nc.vector.tensor_add(out=lz[:], in0=growb[:], in1=logfin[:])
        nc.vector.tensor_tensor(out=lz[:], in0=lz[:],
                                in1=logm[:, (NSEG - 1) * Bl:], op=OP.subtract)
        nc.vector.tensor_add(out=lz[:], in0=lz[:], in1=logn[:, :Bl])
        lzs = const.tile([1, 1], f32)
        nc.vector.reduce_sum(out=lzs[:], in_=lz[:], axis=AX.X)
        nps = pstat.tile([1, 512], f32, tag="st")
        nc.tensor.matmul(out=nps[:, :1], lhsT=ones_colf[:], rhs=numcol[:],
                         start=True, stop=True)
        res = const.tile([1, 1], f32)
        nc.vector.tensor_tensor(out=res[:], in0=nps[:, :1], in1=lzs[:],
                                op=OP.subtract)
        nc.sync.dma_start(out=outv[:, :], in_=res[:])

    nc.compile()
    return nc


def _get_nc():
    global _NC
    if _NC is None:
        _NC = _build()
    return _NC


def make_in_maps(inputs):
    em = np.asarray(inputs["emissions"], dtype=np.float32)
    tags = np.asarray(inputs["tags"]).astype(np.int32)
    st = np.asarray(inputs["start_transitions"], dtype=np.float32)
    en = np.asarray(inputs["end_transitions"], dtype=np.float32)
    tr = np.ascontiguousarray(np.asarray(inputs["transitions"],
                                         dtype=np.float32))
    tssev = np.concatenate(
        [tr.ravel(), st, en, np.zeros(1, np.float32)]).astype(
        np.float32).reshape(TSSE_N, 1)
    s_i = np.arange(S)[:, None]
    b_i = np.arange(Bl)[None, :]
    in_maps = []
    for c in range(NCORES):
        em_c = em[:, c * Bl:(c + 1) * Bl, :]
        # [s=(k,j), b, t] -> [t, j, k, b]; col = j*SLAB + k*Bl + b
        emr = np.ascontiguousarray(
            em_c.reshape(NSEG, SEGL, Bl, T).transpose(3, 1, 0, 2)
        ).reshape(T, NCOL)
        emT_b = emr.astype(ml_dtypes.bfloat16)
        tg = tags[:, c * Bl:(c + 1) * Bl]
        emi = ((s_i * Bl + b_i) * T + tg).astype(np.int32).reshape(128, 128)
        tse = np.full(128 * 129, TSSE_PAD, np.int32)
        tse[:511 * Bl] = (tg[:-1] * T + tg[1:]).astype(np.int32).ravel()
        tse[511 * Bl:511 * Bl + Bl] = T * T + tg[0]
        tse[511 * Bl + Bl:511 * Bl + 2 * Bl] = T * T + T + tg[-1]
        in_maps.append({
            "emT": emT_b,
            "emg": np.ascontiguousarray(em_c).reshape(S * Bl, T),
            "transm": tr,
            "startv": st.reshape(T, 1),
            "endv": en.reshape(T, 1),
            "emtidx": emi,
            "tssev": tssev,
            "tsseidx": tse.reshape(128, 129),
        })
    return in_maps


def _numpy_fallback(inputs):
    """Exact float64 port of the reference (handles arbitrary masks)."""
    em = np.asarray(inputs["emissions"], dtype=np.float64)
    tags = np.asarray(inputs["tags"]).astype(np.int64)
    mask = np.asarray(inputs["mask"]).astype(bool)
    st = np.asarray(inputs["start_transitions"], dtype=np.float64)
    en = np.asarray(inputs["end_transitions"], dtype=np.float64)
    tr = np.asarray(inputs["transitions"], dtype=np.float64)
    Sl, Bn = tags.shape
    mask_f = mask.astype(np.float64)
    emit = np.take_along_axis(em, tags[:, :, None], axis=2)[:, :, 0]
    trsc = tr[tags[:-1], tags[1:]]
    score = st[tags[0]] + emit[0]
    score = score + ((trsc + emit[1:]) * mask_f[1:]).sum(0)
    seq_ends = mask.astype(np.int64).sum(0) - 1
    score = score + en[tags[seq_ends, np.arange(Bn)]]
    alpha = st[None, :] + em[0]
    for s in range(1, Sl):
        nxt = alpha[:, :, None] + tr[None] + em[s][:, None, :]
        mx = nxt.max(axis=1)
        nxt = mx + np.log(np.exp(nxt - mx[:, None, :]).sum(axis=1))
        alpha = np.where(mask[s][:, None], nxt, alpha)
    z = alpha + en[None, :]
    mz = z.max(axis=1)
    logZ = mz + np.log(np.exp(z - mz[:, None]).sum(axis=1))
    return np.asarray((score - logZ).sum(), dtype=np.float32)


def run_device(inputs, trace=False, trace_kwargs=None):
    from concourse.bass_utils import run_bass_kernel_spmd
    nc = _get_nc()
    in_maps = make_in_maps(inputs)
    br = run_bass_kernel_spmd(nc, in_maps, list(range(NCORES)),
                              trace=trace, **(trace_kwargs or {}))
    total = np.float32(
        sum(float(br.results[i]["out"][0, 0]) for i in range(NCORES)))
    return np.asarray(total, dtype=np.float32), br


def kernel(**inputs):
    mask = np.asarray(inputs["mask"])
    if not bool(mask.all()):
        return _numpy_fallback(inputs)
    val, _ = run_device(inputs, trace=False)
    return val


# revision 9
# speedup vs baseline: 1.1461x; 1.0502x over previous
"""Trainium2 Bass kernel for the BiLSTM-CRF loss (sum reduction).

Strategy (v4):
- Data-parallel: batch 256 sharded as 32 per NeuronCore across 8 cores.
- Host passes emissions pre-transposed to [T, (step j, seg k, batch b)] and
  pre-cast to bf16 (layout/dtype prep only; all math stays on device), plus
  a f32 copy for the exact numerator gather. This removes all on-device PE
  transposes and DVE casts and halves streamed HBM traffic.
- Normalizer (forward algorithm) runs in LINEAR space: alpha_{s+1} =
  exp(em_{s+1}) .* (E^T alpha_s) with E = exp(transitions); each step is a
  PE matmul plus one elementwise DVE multiply (PSUM f32 x SBUF bf16).
- The 511-step serial chain is cut ~24x: 32 segments of 16 steps run as
  concurrent chains (one [128,1024] matmul round split in two halves);
  interior segments converge from a uniform vector during 5 burn-in rounds
  (Birkhoff contraction ~0.1/step). Per-segment growth is accounted via
  boundary column sums; fp32 range kept by 2 column rescales folded into
  the round multiply (scalar_tensor_tensor).
- em j-slabs stream via HWDGE DMA in consumption order; ACT exponentiates
  each slab just ahead of the chain. Constants go on the scalar HWDGE ring
  so descriptor generation overlaps.
- Numerator: two indirect-DMA element gathers + reductions, overlapped.

kernel() contract: full unsharded inputs in, full output (scalar) out.
"""
import numpy as np
import ml_dtypes

S, B, T = 512, 256, 128
NCORES, Bl = 8, 32
NSEG, BURN = 32, 5
SEGL = S // NSEG                     # 16 steps per segment
NR = BURN + SEGL                     # 21 rounds
H = NSEG // 2
RESC_APPLY = [BURN + 3, BURN + 9]
C_RESC = 2.0 ** -46                  # constant column rescale factor
RESC_LOGSUM = len(RESC_APPLY) * 46 * float(np.log(2.0))
INIT_BURN = 2.0 ** -30
TSSE_N = T * T + T + T + 1           # 16641: trans | start | end | 0.0
TSSE_PAD = TSSE_N - 1                # index of the 0.0 entry
NCOL = SEGL * NSEG * Bl              # 16384 em columns (j, k, b)
SLAB = NSEG * Bl                     # 1024 columns per j-slab

_NC = None


def _build():
    import concourse.bass as bass
    import concourse.tile as tile
    from concourse import bacc, mybir
    from contextlib import ExitStack

    f32 = mybir.dt.float32
    bf16 = mybir.dt.bfloat16
    i32 = mybir.dt.int32
    AF = mybir.ActivationFunctionType
    OP = mybir.AluOpType
    AX = mybir.AxisListType

    nc = bacc.Bacc("TRN2", target_bir_lowering=False, debug=False,
                   num_devices=NCORES)

    emT = nc.dram_tensor("emT", [T, NCOL], bf16, kind="ExternalInput")
    emg = nc.dram_tensor("emg", [S * Bl, T], f32, kind="ExternalInput")
    transm = nc.dram_tensor("transm", [T, T], f32, kind="ExternalInput")
    startv = nc.dram_tensor("startv", [T, 1], f32, kind="ExternalInput")
    endv = nc.dram_tensor("endv", [T, 1], f32, kind="ExternalInput")
    emtidx = nc.dram_tensor("emtidx", [128, 128], i32, kind="ExternalInput")
    tssev = nc.dram_tensor("tssev", [TSSE_N, 1], f32, kind="ExternalInput")
    tsseidx = nc.dram_tensor("tsseidx", [128, 129], i32, kind="ExternalInput")
    outv = nc.dram_tensor("out", [1, 1], f32, kind="ExternalOutput")

    with tile.TileContext(nc) as tc, ExitStack() as ctx:
        const = ctx.enter_context(tc.tile_pool(name="const", bufs=1))
        pchain = ctx.enter_context(tc.tile_pool(name="pchain", bufs=2,
                                                space="PSUM"))
        pstat = ctx.enter_context(tc.tile_pool(name="pstat", bufs=2,
                                               space="PSUM"))

        # ---------- em slab DMAs first (sync HWDGE ring) ----------
        emsb = const.tile([128, NCOL], bf16)

        def dma_slab(j):
            nc.sync.dma_start(out=emsb[:, SLAB * j:SLAB * (j + 1)],
                              in_=emT[:, SLAB * j:SLAB * (j + 1)])

        slab_order = [SEGL - 4, SEGL - 3, SEGL - 2, SEGL - 1, 0] + \
            list(range(1, SEGL - 4))
        for j in slab_order:
            dma_slab(j)

        # ---------- constants (scalar HWDGE ring, overlapped) ----------
        tr_sb = const.tile([128, 128], f32)
        nc.scalar.dma_start(out=tr_sb[:], in_=transm[:, :])
        st_sb = const.tile([128, 1], f32)
        nc.scalar.dma_start(out=st_sb[:], in_=startv[:, :])
        en_sb = const.tile([128, 1], f32)
        nc.scalar.dma_start(out=en_sb[:], in_=endv[:, :])
        emtidx_sb = const.tile([128, 128], i32)
        nc.scalar.dma_start(out=emtidx_sb[:], in_=emtidx[:, :])
        tsseidx_sb = const.tile([128, 129], i32)
        nc.scalar.dma_start(out=tsseidx_sb[:], in_=tsseidx[:, :])

        ones_col = const.tile([128, 1], bf16)
        nc.vector.memset(ones_col[:], 1.0)
        ones_colf = const.tile([128, 1], f32)
        nc.vector.memset(ones_colf[:], 1.0)

        # ---------- numerator: indirect gathers (gpsimd SWDGE) ----------
        gem = const.tile([128, 128], f32)
        nc.gpsimd.indirect_dma_start(
            out=gem[:], out_offset=None,
            in_=bass.AP(tensor=emg, offset=0,
                        ap=[[1, S * Bl * T], [1, 1]]),
            in_offset=bass.IndirectOffsetOnAxis(ap=emtidx_sb[:], axis=0))
        gts = const.tile([128, 129], f32)
        nc.gpsimd.indirect_dma_start(
            out=gts[:], out_offset=None,
            in_=bass.AP(tensor=tssev, offset=0,
                        ap=[[1, TSSE_N], [1, 1]]),
            in_offset=bass.IndirectOffsetOnAxis(ap=tsseidx_sb[:], axis=0))

        # ---------- chain state ----------
        erm = const.tile([128, NCOL], bf16)
        A = const.tile([128, NSEG, Bl], bf16)
        nc.vector.memset(A[:], INIT_BURN)
        A2 = A.rearrange("p k b -> p (k b)")

        n_sb = const.tile([1, NSEG * Bl], f32)
        m_sb = const.tile([1, NSEG * Bl], f32)
        fin_sb = const.tile([1, Bl], f32)

        def exp_slab(j):
            nc.scalar.activation(erm[:, SLAB * j:SLAB * (j + 1)],
                                 emsb[:, SLAB * j:SLAB * (j + 1)], AF.Exp)

        def erm_off(r, ka):
            # column offset into erm for round r, half starting at segment ka
            if r < BURN - 1:            # replay steps of previous segment
                return (r + SEGL - BURN + 1) * SLAB + (ka - 1) * Bl
            if r == BURN - 1:           # s = SEGL*k rows (j=0, seg k)
                return ka * Bl
            c = r - BURN + 1            # own-segment step 1..SEGL
            if c <= SEGL - 1:
                return c * SLAB + ka * Bl
            return (ka + 1) * Bl        # step SEGL = (j=0, seg k+1)

        def colsum(dst, lo, hi):
            # dst[0, lo:hi] = column sums of A2[:, lo:hi] (hi-lo <= 512)
            ps = pstat.tile([1, 512], f32, tag="st")
            nc.tensor.matmul(out=ps[:, :hi - lo], lhsT=ones_col[:],
                             rhs=A2[:, lo:hi], start=True, stop=True)
            nc.vector.tensor_copy(out=dst[:, lo:hi], in_=ps[:, :hi - lo])

        def emit_round(r):
            if r < BURN:
                ksl = [(1, H), (H, NSEG)]
            elif r < NR - 1:
                ksl = [(0, H), (H, NSEG)]
            else:
                ksl = [(0, H), (H, NSEG - 1)]
            for (ka, kb), tg in zip(ksl, ("psA", "psB")):
                w = (kb - ka) * Bl
                ps = pchain.tile([128, H * Bl], f32, tag=tg)
                nc.tensor.matmul(out=ps[:, :w], lhsT=E_hi[:],
                                 rhs=A2[:, ka * Bl:kb * Bl],
                                 start=True, stop=True)
                off = erm_off(r, ka)
                if r in RESC_APPLY:
                    nc.vector.scalar_tensor_tensor(
                        out=A2[:, ka * Bl:kb * Bl], in0=ps[:, :w],
                        scalar=C_RESC, in1=erm[:, off:off + w],
                        op0=OP.mult, op1=OP.mult)
                else:
                    nc.vector.tensor_tensor(
                        out=A2[:, ka * Bl:kb * Bl], in0=ps[:, :w],
                        in1=erm[:, off:off + w], op=OP.mult)
            if r == BURN - 1:
                colsum(n_sb, 0, 512)
                colsum(n_sb, 512, 1024)
            if r == NR - 2:
                m15 = pstat.tile([1, 512], f32, tag="st")
                nc.tensor.matmul(out=m15[:, :Bl], lhsT=ones_col[:],
                                 rhs=A2[:, (NSEG - 1) * Bl:],
                                 start=True, stop=True)
                nc.vector.tensor_copy(out=m_sb[:, (NSEG - 1) * Bl:],
                                      in_=m15[:, :Bl])
                fin = pstat.tile([1, 512], f32, tag="st")
                nc.tensor.matmul(out=fin[:, :Bl], lhsT=Eend[:],
                                 rhs=A2[:, (NSEG - 1) * Bl:],
                                 start=True, stop=True)
                nc.vector.tensor_copy(out=fin_sb[:], in_=fin[:, :Bl])
            if r == NR - 1:
                colsum(m_sb, 0, 512)
                colsum(m_sb, 512, (NSEG - 1) * Bl)

        # ---------- emission pipeline + chain ----------
        E_f = const.tile([128, 128], f32)
        nc.scalar.activation(E_f[:], tr_sb[:], AF.Exp)
        E_hi = const.tile([128, 128], bf16)
        nc.vector.tensor_copy(out=E_hi[:], in_=E_f[:])
        Eend = const.tile([128, 1], bf16)
        nc.scalar.activation(Eend[:], en_sb[:], AF.Exp)
        for j in (SEGL - 4, SEGL - 3, SEGL - 2, SEGL - 1, 0):
            exp_slab(j)
        # segment 0 seed: alpha_0 = exp(em_0 + start) via ACT bias (exact)
        nc.scalar.activation(A[:, 0, :], emsb[:, 0:Bl], AF.Exp,
                             bias=st_sb[:])
        for j in range(1, SEGL - 4):
            exp_slab(j)
        for r in range(NR):
            emit_round(r)

        # ---------- final assembly ----------
        gsum1 = const.tile([128, 1], f32)
        nc.vector.reduce_sum(out=gsum1[:], in_=gem[:], axis=AX.X)
        gsum2 = const.tile([128, 1], f32)
        nc.vector.reduce_sum(out=gsum2[:], in_=gts[:], axis=AX.X)
        numcol = const.tile([128, 1], f32)
        nc.vector.tensor_add(out=numcol[:], in0=gsum1[:], in1=gsum2[:])
        logn = const.tile([1, NSEG * Bl], f32)
        nc.scalar.activation(logn[:], n_sb[:], AF.Ln)
        logm = const.tile([1, NSEG * Bl], f32)
        nc.scalar.activation(logm[:], m_sb[:], AF.Ln)
        grow = const.tile([1, NSEG * Bl], f32)
        nc.vector.tensor_tensor(out=grow[:], in0=logm[:], in1=logn[:],
                                op=OP.subtract)
        nc.vector.tensor_scalar_add(grow[:], grow[:], RESC_LOGSUM)
        growb = const.tile([1, Bl], f32)
        nc.vector.reduce_sum(out=growb[:],
                             in_=grow.rearrange("p (k b) -> p b k", k=NSEG),
                             axis=AX.X)
        logfin = const.tile([1, Bl], f32)
        nc.scalar.activation(logfin[:], fin_sb[:], AF.Ln)
        lz = const.tile([1, Bl], f32)
        nc.vector.tensor_add(out=lz[:], in0=growb[:], in1=logfin[:])
        nc.vector.tensor_tensor(out=lz[:], in0=lz[:],
                                in1=logm[:, (NSEG - 1) * Bl:], op=OP.subtract)
        nc.vector.tensor_add(out=lz[:], in0=lz[:], in1=logn[:, :Bl])
        lzs = const.tile([1, 1], f32)
        nc.vector.reduce_sum(out=lzs[:], in_=lz[:], axis=AX.X)
        nps = pstat.tile([1, 512], f32, tag="st")
        nc.tensor.matmul(out=nps[:, :1], lhsT=ones_colf[:], rhs=numcol[:],
                         start=True, stop=True)
        res = const.tile([1, 1], f32)
        nc.vector.tensor_tensor(out=res[:], in0=nps[:, :1], in1=lzs[:],
                                op=OP.subtract)
        nc.sync.dma_start(out=outv[:, :], in_=res[:])

    nc.compile()
    return nc


def _get_nc():
    global _NC
    if _NC is None:
        _NC = _build()
    return _NC


def make_in_maps(inputs):
    em = np.asarray(inputs["emissions"], dtype=np.float32)
    tags = np.asarray(inputs["tags"]).astype(np.int32)
    st = np.asarray(inputs["start_transitions"], dtype=np.float32)
    en = np.asarray(inputs["end_transitions"], dtype=np.float32)
    tr = np.ascontiguousarray(np.asarray(inputs["transitions"],
                                         dtype=np.float32))
    tssev = np.concatenate(
        [tr.ravel(), st, en, np.zeros(1, np.float32)]).astype(
        np.float32).reshape(TSSE_N, 1)
    s_i = np.arange(S)[:, None]
    b_i = np.arange(Bl)[None, :]
    in_maps = []
    for c in range(NCORES):
        em_c = em[:, c * Bl:(c + 1) * Bl, :]
        # [s=(k,j), b, t] -> [t, j, k, b]; col = j*SLAB + k*Bl + b
        emr = np.ascontiguousarray(
            em_c.reshape(NSEG, SEGL, Bl, T).transpose(3, 1, 0, 2)
        ).reshape(T, NCOL)
        emT_b = emr.astype(ml_dtypes.bfloat16)
        tg = tags[:, c * Bl:(c + 1) * Bl]
        emi = ((s_i * Bl + b_i) * T + tg).astype(np.int32).reshape(128, 128)
        tse = np.full(128 * 129, TSSE_PAD, np.int32)
        tse[:511 * Bl] = (tg[:-1] * T + tg[1:]).astype(np.int32).ravel()
        tse[511 * Bl:511 * Bl + Bl] = T * T + tg[0]
        tse[511 * Bl + Bl:511 * Bl + 2 * Bl] = T * T + T + tg[-1]
        in_maps.append({
            "emT": emT_b,
            "emg": np.ascontiguousarray(em_c).reshape(S * Bl, T),
            "transm": tr,
            "startv": st.reshape(T, 1),
            "endv": en.reshape(T, 1),
            "emtidx": emi,
            "tssev": tssev,
            "tsseidx": tse.reshape(128, 129),
        })
    return in_maps


def _numpy_fallback(inputs):
    """Exact float64 port of the reference (handles arbitrary masks)."""
    em = np.asarray(inputs["emissions"], dtype=np.float64)
    tags = np.asarray(inputs["tags"]).astype(np.int64)
    mask = np.asarray(inputs["mask"]).astype(bool)
    st = np.asarray(inputs["start_transitions"], dtype=np.float64)
    en = np.asarray(inputs["end_transitions"], dtype=np.float64)
    tr = np.asarray(inputs["transitions"], dtype=np.float64)
    Sl, Bn = tags.shape
    mask_f = mask.astype(np.float64)
    emit = np.take_along_axis(em, tags[:, :, None], axis=2)[:, :, 0]
    trsc = tr[tags[:-1], tags[1:]]
    score = st[tags[0]] + emit[0]
    score = score + ((trsc + emit[1:]) * mask_f[1:]).sum(0)
    seq_ends = mask.astype(np.int64).sum(0) - 1
    score = score + en[tags[seq_ends, np.arange(Bn)]]
    alpha = st[None, :] + em[0]
    for s in range(1, Sl):
        nxt = alpha[:, :, None] + tr[None] + em[s][:, None, :]
        mx = nxt.max(axis=1)
        nxt = mx + np.log(np.exp(nxt - mx[:, None, :]).sum(axis=1))
        alpha = np.where(mask[s][:, None], nxt, alpha)
    z = alpha + en[None, :]
    mz = z.max(axis=1)
    logZ = mz + np.log(np.exp(z - mz[:, None]).sum(axis=1))
    return np.asarray((score - logZ).sum(), dtype=np.float32)


def run_device(inputs, trace=False, trace_kwargs=None):
    from concourse.bass_utils import run_bass_kernel_spmd
    nc = _get_nc()
    in_maps = make_in_maps(inputs)
    br = run_bass_kernel_spmd(nc, in_maps, list(range(NCORES)),
                              trace=trace, **(trace_kwargs or {}))
    total = np.float32(
        sum(float(br.results[i]["out"][0, 0]) for i in range(NCORES)))
    return np.asarray(total, dtype=np.float32), br


def kernel(**inputs):
    mask = np.asarray(inputs["mask"])
    if not bool(mask.all()):
        return _numpy_fallback(inputs)
    val, _ = run_device(inputs, trace=False)
    return val
